# revision 1
# baseline (speedup 1.0000x reference)
"""GQA attention kernel for 8 TRN2 NeuronCores.

Sharding: DP over batch (2) x TP over heads (4 shards): each core gets
4 Q heads + 2 KV heads of one batch. Host pre-transposes/retiles inputs,
device computes QKV proj + QK-RMSNorm + RoPE + causal attention + o_proj
partial; host sums the 4 o_proj partials per batch.

Key device-side layout choices:
  - projections computed in "natural" [s, c] layout (lhsT = x^T tiles,
    rhs = W^T) so RMSNorm/RoPE reduce over the free dim;
  - q/k re-transposed per 128x128 tile on the PE into [hd, s] layout;
  - scores computed transposed sT[k, q] so the exp'd probabilities can be
    used directly as the stationary operand of the PV matmul; q-tiles are
    processed in PAIRS (moving free dim 256) to halve QK instruction count;
  - no max-subtraction in softmax: RMSNorm bounds |q.k|/sqrt(hd) <= 11.32
    so exp() is fp32-safe;
  - softmax denominator comes free from a ones-column appended to V;
  - k+v projections share one PSUM bank, so they are ONE matmul per d-tile
    (start=True clears has_written for the whole bank).

All matmuls run in bf16 (1 PE cycle/row); accumulation is fp32 in PSUM.
"""

import numpy as np
import ml_dtypes

import concourse.bass as bass
import concourse.mybir as mybir
from concourse import bacc
from concourse.tile import TileContext
from concourse.bass_utils import run_bass_kernel_spmd

B, S, D = 2, 2048, 2048
H, KVH, HD = 16, 8, 128
NSH = 4          # TP shards per batch
HLOC = H // NSH  # 4 q heads per core
KVLOC = KVH // NSH
OC = HLOC * HD   # 512 attn-out channels per core
ST = S // 128    # 16 s-tiles
DT = D // 128    # 16 d-tiles
VW = 132         # v row width: 128 hd + 1 ones + 3 pad
SCALE = 1.0 / np.sqrt(HD)

BF16 = mybir.dt.bfloat16
F32 = mybir.dt.float32
AF = mybir.ActivationFunctionType
ALU = mybir.AluOpType

_cache = {}


def build_nc():
    nc = bacc.Bacc()

    xt = nc.declare_dram_parameter("xt", [ST // 2, DT, 128, 256], BF16, isOutput=False)
    wqkv = nc.declare_dram_parameter("wqkv", [DT, 128, 1024], BF16, isOutput=False)
    wot = nc.declare_dram_parameter("wot", [HLOC, 128, D], BF16, isOutput=False)
    qcos = nc.declare_dram_parameter("qcos", [ST, 128, HD], BF16, isOutput=False)
    qsin = nc.declare_dram_parameter("qsin", [ST, 128, HD], BF16, isOutput=False)
    kcos = nc.declare_dram_parameter("kcos", [ST, 128, HD], BF16, isOutput=False)
    ksin = nc.declare_dram_parameter("ksin", [ST, 128, HD], BF16, isOutput=False)
    maskp = nc.declare_dram_parameter("mask", [128, 128], BF16, isOutput=False)
    identp = nc.declare_dram_parameter("ident", [128, 128], BF16, isOutput=False)
    out = nc.declare_dram_parameter("out", [S, D], BF16, isOutput=True)

    with TileContext(nc) as tc:
        with (
            tc.tile_pool(name="const", bufs=1) as constp,
            tc.tile_pool(name="xs", bufs=3) as xsp,
            tc.tile_pool(name="work", bufs=3) as workp,
            tc.tile_pool(name="pt", bufs=2) as ptp,
            tc.tile_pool(name="small", bufs=4) as smallp,
            tc.tile_pool(name="at", bufs=2) as atp,
            tc.tile_pool(name="psA", bufs=2, space="PSUM") as psA,
            tc.tile_pool(name="psS", bufs=2, space="PSUM") as psS,
            tc.tile_pool(name="psO", bufs=2, space="PSUM") as psO,
        ):
            # ---- persistent tiles ----
            w_sb = constp.tile([128, DT, 1024], BF16, tag="w")
            for wg in range(4):
                nc.sync.dma_start(out=w_sb[:, wg * 4:(wg + 1) * 4, :],
                                  in_=wqkv[wg * 4:(wg + 1) * 4].transpose([1, 0, 2]))
            wo_sb = constp.tile([128, HLOC, D], BF16, tag="wo")
            nc.scalar.dma_start(out=wo_sb, in_=wot[:].transpose([1, 0, 2]))
            cs_tiles = {}
            for nm, prm in (("qc", qcos), ("qs", qsin), ("kc", kcos), ("ks", ksin)):
                t = constp.tile([128, ST, HD], BF16, tag=nm)
                nc.gpsimd.dma_start(out=t, in_=prm[:].transpose([1, 0, 2]))
                cs_tiles[nm] = t
            mask_sb = constp.tile([128, 128], BF16, tag="mask")
            nc.scalar.dma_start(out=mask_sb, in_=maskp[:])
            id_sb = constp.tile([128, 128], BF16, tag="ident")
            nc.scalar.dma_start(out=id_sb, in_=identp[:])

            qT = constp.tile([128, HLOC, S], BF16, tag="qT")
            kT = constp.tile([128, KVLOC, S], BF16, tag="kT")
            v_sb = constp.tile([128, ST, KVLOC, VW], BF16, tag="v")
            nc.gpsimd.memset(v_sb, 0.0)
            nc.gpsimd.memset(v_sb[:, :, :, 128:129], 1.0)

            def phase1(i, xs):
                """qkv proj + rmsnorm + rope + transpose for s-tile i"""
                half = slice((i % 2) * 128, (i % 2) * 128 + 128)
                pq = psA.tile([128, 1024], F32, tag="pq")
                for dt in range(DT):
                    lhsT = xs[:, dt, half]
                    st, sp = dt == 0, dt == DT - 1
                    nc.tensor.matmul(pq[:, 0:512], lhsT, w_sb[:, dt, 0:512],
                                     start=st, stop=sp)
                    nc.tensor.matmul(pq[:, 512:1024], lhsT, w_sb[:, dt, 512:1024],
                                     start=st, stop=sp)

                # rms scales for 6 sub-heads (4 q + 2 k)
                ssq = smallp.tile([128, 8], F32, tag="ssq")
                sqs = workp.tile([128, 128], F32, tag="sqs")
                for c in range(6):
                    nc.scalar.activation(sqs, pq[:, c * 128:(c + 1) * 128], AF.Square,
                                         accum_out=ssq[:, c:c + 1])
                tm = smallp.tile([128, 8], F32, tag="tm")
                nc.scalar.activation(tm[:, 0:6], ssq[:, 0:6], AF.Sqrt, scale=1.0 / HD)
                scl = smallp.tile([128, 8], F32, tag="scl")
                nc.vector.reciprocal(scl[:, 0:6], tm[:, 0:6])

                qh = workp.tile([128, 6, 128], BF16, tag="qh")
                nc.vector.tensor_mul(qh[:, 0:4], pq[:, 0:512].rearrange("p (h d) -> p h d", h=4),
                                     scl[:, 0:4].unsqueeze(2).broadcast_to([128, 4, 128]))
                nc.vector.tensor_mul(qh[:, 4:6], pq[:, 512:768].rearrange("p (h d) -> p h d", h=2),
                                     scl[:, 4:6].unsqueeze(2).broadcast_to([128, 2, 128]))
                # v: psum -> sbuf bf16
                nc.vector.tensor_copy(v_sb[:, i, :, 0:128],
                                      pq[:, 768:1024].rearrange("p (kv hd) -> p kv hd", kv=2))

                # rope: rp = qh*cosW + swap(qh)*sinW (signs/norm-w folded into tables)
                t1 = workp.tile([128, 6, 128], BF16, tag="t1")
                t2 = workp.tile([128, 6, 128], BF16, tag="t2")
                rp = workp.tile([128, 6, 128], BF16, tag="rp")
                for lo, hi, pre in ((0, 4, "q"), (4, 6, "k")):
                    n = hi - lo
                    cosT = cs_tiles[pre + "c"][:, i, :].unsqueeze(1)
                    sinT = cs_tiles[pre + "s"][:, i, :].unsqueeze(1)
                    nc.vector.tensor_mul(t1[:, lo:hi], qh[:, lo:hi],
                                         cosT.broadcast_to([128, n, 128]))
                    nc.vector.tensor_mul(t2[:, lo:hi, 0:64], qh[:, lo:hi, 64:128],
                                         sinT[:, :, 0:64].broadcast_to([128, n, 64]))
                    nc.vector.tensor_mul(t2[:, lo:hi, 64:128], qh[:, lo:hi, 0:64],
                                         sinT[:, :, 64:128].broadcast_to([128, n, 64]))
                    nc.vector.tensor_add(rp[:, lo:hi], t1[:, lo:hi], t2[:, lo:hi])

                # transpose q/k tiles into [hd, s] layout (one psum tile, 2 copies)
                ptt = psO.tile([128, 768], BF16, tag="po")
                for c in range(6):
                    nc.tensor.transpose(ptt[:, c * 128:(c + 1) * 128], rp[:, c], id_sb)
                nc.vector.tensor_copy(qT[:, :, i * 128:(i + 1) * 128],
                                      ptt[:, 0:512].rearrange("p (h d) -> p h d", h=4))
                nc.vector.tensor_copy(kT[:, :, i * 128:(i + 1) * 128],
                                      ptt[:, 512:768].rearrange("p (h d) -> p h d", h=2))

            for m in range(ST // 2):
                i0, i1 = 2 * m, 2 * m + 1
                xs = xsp.tile([128, DT, 256], BF16, tag="xs")
                nc.sync.dma_start(out=xs, in_=xt[m].transpose([1, 0, 2]))
                phase1(i0, xs)
                phase1(i1, xs)

                # ---- attention for q-tile pair (i0, i1) ----
                aT = atp.tile([128, 2, HLOC, 128], BF16, tag="aT")
                for h in range(HLOC):
                    kv = h // 2
                    pT = ptp.tile([128, 2 * S], BF16, tag="pT")  # [k, (j, 2q)]
                    # QK: moving = 256 q columns (both tiles), j = 0..i1
                    for g in range(m + 1):
                        ps = psS.tile([128, 512], F32, tag="ps")
                        for jj in range(2):
                            j = 2 * g + jj
                            nc.tensor.matmul(ps[:, jj * 256:(jj + 1) * 256],
                                             kT[:, kv, j * 128:(j + 1) * 128],
                                             qT[:, h, i0 * 128:i0 * 128 + 256],
                                             start=True, stop=True)
                        nc.scalar.activation(pT[:, g * 512:(g + 1) * 512], ps,
                                             AF.Exp, scale=SCALE)
                    # causal masks on the two diagonal tiles
                    nc.vector.tensor_mul(pT[:, i0 * 256:i0 * 256 + 128],
                                         pT[:, i0 * 256:i0 * 256 + 128], mask_sb)
                    nc.vector.tensor_mul(pT[:, i1 * 256 + 128:i1 * 256 + 256],
                                         pT[:, i1 * 256 + 128:i1 * 256 + 256], mask_sb)
                    ob = workp.tile([128, 2, 128], BF16, tag="ob")
                    for t in range(2):
                        i = i0 + t
                        po = psO.tile([128, 132], F32, tag="po")
                        for j in range(i + 1):
                            nc.tensor.matmul(po,
                                             pT[:, j * 256 + t * 128:j * 256 + t * 128 + 128],
                                             v_sb[:, j, kv, :],
                                             start=(j == 0), stop=(j == i))
                        rcp = smallp.tile([128, 1], F32, tag="rcp")
                        nc.vector.reciprocal(rcp, po[:, 128:129])
                        nc.vector.tensor_mul(ob[:, t], po[:, 0:128],
                                             rcp.broadcast_to([128, 128]))
                    ptt = psO.tile([128, 768], BF16, tag="po")
                    for t in range(2):
                        nc.tensor.transpose(ptt[:, t * 128:(t + 1) * 128], ob[:, t], id_sb)
                    nc.vector.tensor_copy(aT[:, :, h, :],
                                          ptt[:, 0:256].rearrange("p (t d) -> p t d", t=2))

                # ---- o_proj partials for s-tiles i0, i1 ----
                for t in range(2):
                    i = i0 + t
                    ob2 = workp.tile([128, D], BF16, tag="ob2")
                    for n in range(4):
                        pp = psS.tile([128, 512], F32, tag="ps")
                        for ct in range(HLOC):
                            nc.tensor.matmul(pp, aT[:, t, ct, :],
                                             wo_sb[:, ct, n * 512:(n + 1) * 512],
                                             start=(ct == 0), stop=(ct == HLOC - 1))
                        nc.vector.tensor_copy(ob2[:, n * 512:(n + 1) * 512], pp)
                    nc.sync.dma_start(out=out[i * 128:(i + 1) * 128, :], in_=ob2)
    nc.finalize()
    return nc


def _host_prep(hidden_states, Wq, Wk, Wv, Wo, q_norm_w, k_norm_w):
    bf = ml_dtypes.bfloat16
    inv_freq = 1.0 / (10000.0 ** (np.arange(0, HD, 2, dtype=np.float64) / HD))
    pos = np.arange(S, dtype=np.float64)
    freqs = np.outer(pos, inv_freq)                       # [S, 64]
    emb = np.concatenate([freqs, freqs], axis=-1)         # [S, 128]
    cos = np.cos(emb).astype(np.float32)
    sin = np.sin(emb).astype(np.float32)

    def fold(w):
        w = np.asarray(w, np.float32)
        cosw = (cos * w[None, :]).astype(bf)
        swapsign = np.concatenate([-w[64:], w[:64]])
        sinw = (sin * swapsign[None, :]).astype(bf)
        return cosw.reshape(ST, 128, HD), sinw.reshape(ST, 128, HD)

    qc, qs = fold(q_norm_w)
    kc, ks = fold(k_norm_w)

    mask = np.triu(np.ones((128, 128), np.float32)).astype(bf)   # [k,q] keep q>=k
    ident = np.eye(128, dtype=np.float32).astype(bf)

    in_maps = []
    for core in range(8):
        b, sh = core // NSH, core % NSH
        xT = np.ascontiguousarray(hidden_states[b].T).astype(bf)     # [D, S]
        xt = np.ascontiguousarray(
            xT.reshape(DT, 128, ST // 2, 256).transpose(2, 0, 1, 3))  # [8,16,128,256]
        wq = Wq[sh * OC:(sh + 1) * OC]                                # [512, D]
        wk = Wk[sh * KVLOC * HD:(sh + 1) * KVLOC * HD]                # [256, D]
        wv = Wv[sh * KVLOC * HD:(sh + 1) * KVLOC * HD]
        wcat = np.concatenate([wq, wk, wv], axis=0)                   # [1024, D]
        wqkv = np.ascontiguousarray(wcat.T.astype(bf).reshape(DT, 128, 1024))
        wotn = np.ascontiguousarray(
            Wo[:, sh * OC:(sh + 1) * OC].T.astype(bf).reshape(HLOC, 128, D))
        in_maps.append({
            "xt": xt, "wqkv": wqkv, "wot": wotn,
            "qcos": qc, "qsin": qs, "kcos": kc, "ksin": ks,
            "mask": mask, "ident": ident,
        })
    return in_maps


def run(in_maps, **kw):
    if "nc" not in _cache:
        _cache["nc"] = build_nc()
    return run_bass_kernel_spmd(_cache["nc"], in_maps, core_ids=list(range(8)), **kw)


def kernel(**inputs):
    in_maps = _host_prep(**inputs)
    res = run(in_maps).results
    out = np.zeros((B, S, D), np.float32)
    for core in range(8):
        out[core // NSH] += res[core]["out"]
    return out



# revision 12
# speedup vs baseline: 1.2808x; 1.2808x over previous
"""GQA attention kernel for 8 TRN2 NeuronCores.

Sharding: DP over batch (2) x TP over heads (4 shards): each core gets
4 Q heads + 2 KV heads of one batch. Host pre-transposes/retiles inputs,
device computes QKV proj + QK-RMSNorm + RoPE + causal attention + o_proj
partial; host sums the 4 o_proj partials per batch.

v2 structure (vs v1): two device-side phases.
  Phase A (per s-tile): QKV proj -> RMSNorm -> RoPE -> PE transpose of
    q/k into [hd, s] layout. RMSNorm sum-of-squares is ONE scalar Square
    + ONE vector segmented reduce (v1 used 6 Square+ReadAcc pairs per
    tile). RoPE tables are pre-broadcast per sub-head on host so the DVE
    multiplies run on plain contiguous bf16 APs (4x mode). Transposes
    are software-pipelined one s-tile behind the projection matmuls.
  Phase B (per q-tile pair): QK -> exp -> PV -> o_proj of the previous
    pair interleaved for PE gap-filling. Exp runs on 1024-col PSUM
    chunks (v1: 512) to amortize the ~352-cycle ACT fixed cost.
  Keeping Square/Sqrt (phase A) and Exp (phase B) in disjoint phases
  avoids v1's 19 ACT table reloads (~29us of scalar churn).

All DRAM parameters are laid out so every DMA is contiguous per
partition (v1's transposing DMAs ran at ~25GB/s and serialized 42us of
startup). Big loads are spread across the sync/scalar/gpsimd queues.

Other invariants from v1:
  - scores computed transposed sT[k, q]; exp'd probabilities feed the
    PV matmul as the stationary operand; q-tiles processed in PAIRS;
  - no max-subtraction in softmax: RMSNorm bounds |q.k|/sqrt(hd) <= 11.3
    so exp() is fp32-safe;
  - softmax denominator from a ones-column appended to V;
  - all matmuls bf16 (1 PE cycle/row); accumulation fp32 in PSUM.
"""

import numpy as np
import ml_dtypes

import concourse.bass as bass
import concourse.mybir as mybir
from concourse import bacc
from concourse.tile import TileContext
from concourse.bass_utils import run_bass_kernel_spmd

B, S, D = 2, 2048, 2048
H, KVH, HD = 16, 8, 128
NSH = 4          # TP shards per batch
HLOC = H // NSH  # 4 q heads per core
KVLOC = KVH // NSH
OC = HLOC * HD   # 512 attn-out channels per core
ST = S // 128    # 16 s-tiles
DT = D // 128    # 16 d-tiles
VW = 132         # v row width: 128 hd + 1 ones + 3 pad
SCALE = 1.0 / np.sqrt(HD)
EPS = 1e-6

BF16 = mybir.dt.bfloat16
F32 = mybir.dt.float32
AF = mybir.ActivationFunctionType
ALU = mybir.AluOpType

_cache = {}


def build_nc():
    nc = bacc.Bacc()

    # all params partition-major so DMAs are contiguous per partition
    xt = nc.declare_dram_parameter("xt", [ST // 2, 128, DT, 256], BF16, isOutput=False)
    wqkv = nc.declare_dram_parameter("wqkv", [4, 128, 4, 1024], BF16, isOutput=False)
    wot = nc.declare_dram_parameter("wot", [128, HLOC, D], BF16, isOutput=False)
    ropc = nc.declare_dram_parameter("ropc", [4, 128, 4, 768], BF16, isOutput=False)
    rops = nc.declare_dram_parameter("rops", [4, 128, 4, 768], BF16, isOutput=False)
    maskp = nc.declare_dram_parameter("mask", [128, 128], BF16, isOutput=False)
    identp = nc.declare_dram_parameter("ident", [128, 128], BF16, isOutput=False)
    out = nc.declare_dram_parameter("out", [S, D], BF16, isOutput=True)

    with TileContext(nc) as tc:
        with (
            tc.tile_pool(name="const", bufs=1) as constp,
            tc.tile_pool(name="xs", bufs=2) as xsp,
            tc.tile_pool(name="work", bufs=2) as workp,
            tc.tile_pool(name="sq", bufs=2) as sqp,
            tc.tile_pool(name="pt", bufs=2) as ptp,
            tc.tile_pool(name="small", bufs=4) as smallp,
            tc.tile_pool(name="at", bufs=2) as atp,
            tc.tile_pool(name="ob2", bufs=2) as ob2p,
        ):

            # ---- persistent tiles / constant loads ----
            # critical-path order: w group 0 + xs[0] gate the first matmul
            w_sb = constp.tile([128, DT, 1024], BF16, tag="w")
            nc.sync.dma_start(out=w_sb[:, 0:4, :], in_=wqkv[0])
            mask_sb = constp.tile([128, 128], BF16, tag="mask")
            nc.scalar.dma_start(out=mask_sb, in_=maskp[:])
            id_sb = constp.tile([128, 128], BF16, tag="ident")
            nc.scalar.dma_start(out=id_sb, in_=identp[:])
            cosw = constp.tile([128, ST, 768], BF16, tag="cosw")
            sinw = constp.tile([128, ST, 768], BF16, tag="sinw")
            for g in range(4):
                nc.scalar.dma_start(out=cosw[:, g * 4:(g + 1) * 4, :], in_=ropc[g])
                nc.scalar.dma_start(out=sinw[:, g * 4:(g + 1) * 4, :], in_=rops[g])
            for g in range(1, 4):
                nc.sync.dma_start(out=w_sb[:, g * 4:(g + 1) * 4, :], in_=wqkv[g])
            wo_sb = constp.tile([128, HLOC, D], BF16, tag="wo")
            nc.sync.dma_start(out=wo_sb, in_=wot[:])

            qT = constp.tile([128, HLOC, S], BF16, tag="qT")
            kT = constp.tile([128, KVLOC, S], BF16, tag="kT")
            v_sb = constp.tile([128, ST, KVLOC, VW], BF16, tag="v")

            # ================= PHASE A =================
            def proj(i, xs, psA):
                """qkv projection matmuls for s-tile i -> psum pq"""
                half = slice((i % 2) * 128, (i % 2) * 128 + 128)
                pq = psA.tile([128, 1024], F32, tag="pq")
                for dt in range(DT):
                    lhsT = xs[:, dt, half]
                    st, sp = dt == 0, dt == DT - 1
                    nc.tensor.matmul(pq[:, 0:512], lhsT, w_sb[:, dt, 0:512],
                                     start=st, stop=sp)
                    nc.tensor.matmul(pq[:, 512:1024], lhsT, w_sb[:, dt, 512:1024],
                                     start=st, stop=sp)
                return pq

            def normrope(i, pq):
                """rmsnorm + rope for s-tile i; returns rp (rotated q/k)"""
                # sum of squares: one Square (scalar) + one segmented
                # reduce (vector)
                sqs = sqp.tile([128, 768], F32, tag="sqs")
                nc.scalar.activation(sqs, pq[:, 0:768], AF.Square)
                ssq = smallp.tile([128, 8], F32, tag="ssq")
                nc.vector.tensor_reduce(ssq[:, 0:6],
                                        sqs.rearrange("p (c d) -> p c d", c=6),
                                        axis=mybir.AxisListType.X, op=ALU.add)
                tm = smallp.tile([128, 8], F32, tag="tm")
                nc.scalar.activation(tm[:, 0:6], ssq[:, 0:6], AF.Sqrt,
                                     scale=1.0 / HD)
                scl = smallp.tile([128, 8], F32, tag="scl")
                nc.vector.reciprocal(scl[:, 0:6], tm[:, 0:6])

                # normalize q+k in one strided mul; v copied on scalar
                qh = workp.tile([128, 6, 128], BF16, tag="qh")
                nc.vector.tensor_mul(qh, pq[:, 0:768].rearrange("p (c d) -> p c d", c=6),
                                     scl[:, 0:6].unsqueeze(2).broadcast_to([128, 6, 128]))
                nc.scalar.copy(v_sb[:, i, :, 0:128],
                               pq[:, 768:1024].rearrange("p (kv hd) -> p kv hd", kv=2))

                # rope on pre-broadcast tables (plain contiguous bf16 APs)
                cw = cosw[:, i, :].rearrange("p (c d) -> p c d", c=6)
                sw = sinw[:, i, :].rearrange("p (c d) -> p c d", c=6)
                t1 = workp.tile([128, 6, 128], BF16, tag="t1")
                t2 = workp.tile([128, 6, 128], BF16, tag="t2")
                rp = workp.tile([128, 6, 128], BF16, tag="rp")
                nc.vector.tensor_mul(t1, qh, cw)
                nc.vector.tensor_mul(t2[:, :, 0:64], qh[:, :, 64:128], sw[:, :, 0:64])
                nc.vector.tensor_mul(t2[:, :, 64:128], qh[:, :, 0:64], sw[:, :, 64:128])
                nc.vector.tensor_add(rp, t1, t2)
                return rp

            def xpose(i, rp, psT):
                """transpose q/k of s-tile i into [hd, s] layout"""
                ptt = psT.tile([128, 768], BF16, tag="ptt")
                for c in range(6):
                    nc.tensor.transpose(ptt[:, c * 128:(c + 1) * 128], rp[:, c], id_sb)
                nc.vector.tensor_copy(qT[:, :, i * 128:(i + 1) * 128],
                                      ptt[:, 0:512].rearrange("p (h d) -> p h d", h=4))
                nc.scalar.copy(kT[:, :, i * 128:(i + 1) * 128],
                               ptt[:, 512:768].rearrange("p (h d) -> p h d", h=2))

            with (
                tc.tile_pool(name="psA", bufs=2, space="PSUM") as psA,
                tc.tile_pool(name="psT", bufs=2, space="PSUM") as psT,
            ):
                pending = None  # software-pipeline transposes one tile behind
                for m in range(ST // 2):
                    xs = xsp.tile([128, DT, 256], BF16, tag="xs")
                    nc.gpsimd.dma_start(out=xs, in_=xt[m])
                    for t in range(2):
                        i = 2 * m + t
                        pq = proj(i, xs, psA)
                        if pending is not None:
                            xpose(*pending, psT)
                        rp = normrope(i, pq)
                        pending = (i, rp)
                xpose(*pending, psT)

            # ================= PHASE B =================
            def attn_head(m, h, aT, psS, psO):
                """QK + exp + PV for head h, q-tile pair (2m, 2m+1)"""
                i0, i1 = 2 * m, 2 * m + 1
                kv = h // 2
                pT = ptp.tile([128, 2 * S], BF16, tag="pT")
                for gg in range(0, m + 1, 2):
                    w = min(2, m + 1 - gg)      # g-groups in this chunk
                    ps = psS.tile([128, 1024], F32, tag="ps")
                    for jj in range(2 * w):
                        j = 2 * gg + jj
                        nc.tensor.matmul(ps[:, jj * 256:(jj + 1) * 256],
                                         kT[:, kv, j * 128:(j + 1) * 128],
                                         qT[:, h, i0 * 128:i0 * 128 + 256],
                                         start=True, stop=True)
                    nc.scalar.activation(pT[:, gg * 512:gg * 512 + 512 * w],
                                         ps[:, 0:512 * w], AF.Exp, scale=SCALE)
                # causal masks on the two diagonal tiles
                nc.vector.tensor_mul(pT[:, i0 * 256:i0 * 256 + 128],
                                     pT[:, i0 * 256:i0 * 256 + 128], mask_sb)
                nc.vector.tensor_mul(pT[:, i1 * 256 + 128:i1 * 256 + 256],
                                     pT[:, i1 * 256 + 128:i1 * 256 + 256], mask_sb)
                ob = workp.tile([128, 2, 128], BF16, tag="ob")
                for t in range(2):
                    i = i0 + t
                    po = psO.tile([128, 132], F32, tag="po")
                    for j in range(i + 1):
                        nc.tensor.matmul(po,
                                         pT[:, j * 256 + t * 128:j * 256 + t * 128 + 128],
                                         v_sb[:, j, kv, :],
                                         start=(j == 0), stop=(j == i))
                    rcp = smallp.tile([128, 1], F32, tag="rcp")
                    nc.vector.reciprocal(rcp, po[:, 128:129])
                    nc.vector.tensor_mul(ob[:, t], po[:, 0:128],
                                         rcp.broadcast_to([128, 128]))
                ptt = psO.tile([128, 256], BF16, tag="po")
                for t in range(2):
                    nc.tensor.transpose(ptt[:, t * 128:(t + 1) * 128], ob[:, t], id_sb)
                nc.vector.tensor_copy(aT[:, :, h, :],
                                      ptt.rearrange("p (t d) -> p t d", t=2))

            def oproj_slot(aTp, n, ob2, psP):
                """o_proj n-th 512-col slab for the previous q-tile pair"""
                for t in range(2):
                    pp = psP.tile([128, 512], F32, tag="pp")
                    for ct in range(HLOC):
                        nc.tensor.matmul(pp, aTp[:, t, ct, :],
                                         wo_sb[:, ct, n * 512:(n + 1) * 512],
                                         start=(ct == 0), stop=(ct == HLOC - 1))
                    nc.vector.tensor_copy(ob2[:, t, n * 512:(n + 1) * 512], pp)

            # ones column + pad for the PV denominator trick (gpsimd is
            # idle here; phase A only writes v_sb cols 0:128)
            nc.gpsimd.memset(v_sb[:, :, :, 128:132], 0.0)
            nc.gpsimd.memset(v_sb[:, :, :, 128:129], 1.0)

            outv = out.rearrange("(i p) d -> p i d", p=128)
            with (
                tc.tile_pool(name="psS", bufs=2, space="PSUM") as psS,
                tc.tile_pool(name="psO", bufs=2, space="PSUM") as psO,
                tc.tile_pool(name="psP", bufs=2, space="PSUM") as psP,
            ):
                aTprev = None
                for m in range(ST // 2):
                    aT = atp.tile([128, 2, HLOC, 128], BF16, tag="aT")
                    if m > 0:
                        ob2 = ob2p.tile([128, 2, D], BF16, tag="ob2", name="ob2")
                    else:
                        ob2 = None
                    for h in range(HLOC):
                        attn_head(m, h, aT, psS, psO)
                        if m > 0:
                            oproj_slot(aTprev, h, ob2, psP)
                    if m > 0:
                        nc.sync.dma_start(
                            out=outv[:, 2 * (m - 1):2 * m, :], in_=ob2)
                    aTprev = aT
                ob2 = ob2p.tile([128, 2, D], BF16, tag="ob2")
                for n in range(4):
                    oproj_slot(aTprev, n, ob2, psP)
                nc.sync.dma_start(out=outv[:, ST - 2:ST, :], in_=ob2)
    nc.finalize()
    return nc


def _host_prep(hidden_states, Wq, Wk, Wv, Wo, q_norm_w, k_norm_w):
    bf = ml_dtypes.bfloat16
    inv_freq = 1.0 / (10000.0 ** (np.arange(0, HD, 2, dtype=np.float64) / HD))
    pos = np.arange(S, dtype=np.float64)
    freqs = np.outer(pos, inv_freq)                       # [S, 64]
    emb = np.concatenate([freqs, freqs], axis=-1)         # [S, 128]
    cos = np.cos(emb).astype(np.float32)                  # [S, 128]
    sin = np.sin(emb).astype(np.float32)

    def fold(w):
        w = np.asarray(w, np.float32)
        cosw = cos * w[None, :]
        swapsign = np.concatenate([-w[64:], w[:64]])
        sinw = sin * swapsign[None, :]
        return cosw, sinw

    qc, qs = fold(q_norm_w)
    kc, ks = fold(k_norm_w)
    # pre-broadcast per sub-head: [S, 6, 128] -> [4, 128, 4, 768]
    cos6 = np.stack([qc] * HLOC + [kc] * KVLOC, axis=1)
    sin6 = np.stack([qs] * HLOC + [ks] * KVLOC, axis=1)

    def retile(a):  # [S, 6, 128] -> [ST, 128, 768] -> [4, 128, 4, 768]
        a = a.reshape(ST, 128, 768).astype(bf)
        return np.ascontiguousarray(a.reshape(4, 4, 128, 768).transpose(0, 2, 1, 3))

    ropc, rops = retile(cos6), retile(sin6)

    mask = np.triu(np.ones((128, 128), np.float32)).astype(bf)   # [k,q] keep q>=k
    ident = np.eye(128, dtype=np.float32).astype(bf)

    in_maps = []
    for core in range(8):
        b, sh = core // NSH, core % NSH
        xT = np.ascontiguousarray(hidden_states[b].T).astype(bf)     # [D, S]
        # [D,S] -> [DT,128,ST//2,256] -> [ST//2, 128, DT, 256]
        xt = np.ascontiguousarray(
            xT.reshape(DT, 128, ST // 2, 256).transpose(2, 1, 0, 3))
        wq = Wq[sh * OC:(sh + 1) * OC]                                # [512, D]
        wk = Wk[sh * KVLOC * HD:(sh + 1) * KVLOC * HD]                # [256, D]
        wv = Wv[sh * KVLOC * HD:(sh + 1) * KVLOC * HD]
        wcat = np.concatenate([wq, wk, wv], axis=0)                   # [1024, D]
        # W^T [D, 1024] -> [DT, 128, 1024] -> [4, 128, 4, 1024]
        wqkv = np.ascontiguousarray(
            wcat.T.astype(bf).reshape(4, 4, 128, 1024).transpose(0, 2, 1, 3))
        # Wo cols for this shard: [D, 512] -> T [512, D] -> [128, HLOC, D]
        wotn = np.ascontiguousarray(
            Wo[:, sh * OC:(sh + 1) * OC].T.astype(bf)
            .reshape(HLOC, 128, D).transpose(1, 0, 2))
        in_maps.append({
            "xt": xt, "wqkv": wqkv, "wot": wotn,
            "ropc": ropc, "rops": rops,
            "mask": mask, "ident": ident,
        })
    return in_maps


def run(in_maps, **kw):
    if "nc" not in _cache:
        _cache["nc"] = build_nc()
    return run_bass_kernel_spmd(_cache["nc"], in_maps, core_ids=list(range(8)), **kw)


def kernel(**inputs):
    in_maps = _host_prep(**inputs)
    res = run(in_maps).results
    out = np.zeros((B, S, D), np.float32)
    for core in range(8):
        out[core // NSH] += res[core]["out"]
    return out
